# revision 14
# baseline (speedup 1.0000x reference)
"""Single-head attention (B=4, S=2048, D=H=1024) on 8 TRN2 NeuronCores.

Core c -> batch c//2, query-half c%2 (QH=1024 query rows per core).

Two algebraic restructurings remove both weight applications from the
sequence dimension:

1. scores = Q@K^T = x (Wq Wk^T) x^T + bias terms. With M = Wq Wk^T
   precomputed on host, scores^T[k,q] = (x M x^T)^T + c[k] + (terms
   constant in k, which cancel in softmax). c[k] = x[k]·(Wk bq) is
   host-precomputed and becomes the per-partition bias of the exp
   activation. Kills the K projection entirely.
2. out = (E@V)/den with V = x@Wv + bv  =>  out = (E@x)@Wv/den + bv.
   GT[d,q] = sum_k x[k,d] E[k,q] comes out of the PE in exactly the
   layout the second matmul needs as stationary (no transposes), Wv is
   applied to 1024 q-rows instead of 2048 k-rows, bv folds into the
   final normalize (scalar_tensor_tensor), and no V exchange / no
   collective is needed at all (pair-AllGather measured ~80us
   door-to-done here - far worse than restructuring it away).

fp8 (e4m3) DoubleRow matmuls contract 256/instruction (2x bf16) where
1-term quantization noise fits the 2e-2 gate (numpy bit-sim 1.577e-2,
HW matched sim to ~4e-6 in every round):
  PT8[d,q] = fp8(2^-10 sum_e M8[e,d] xq8[e,q])     fp8 DR   13.7us
  ST[k,q]  = sum_d xf8[d,k] PT8[d,q]               fp8 DR   27.3us
  ET       = exp(2^-16 ST + cb)  (ACT -> bf16)
  GT[d,q]  = sum_k xrow[k,d] ET[k,q]               bf16     54.6us
  O[q,h]   = sum_d GT[d,q] Wv[d,h]                 bf16     27.3us
  den      = ET^T @ ones                           bf16     ~11us
  out      = O*recip(den) + bv                     (DVE STT)
"""

import os

import numpy as np
import ml_dtypes

B, S, D, H = 4, 2048, 1024, 1024
NCORES = 8
PT = 128            # partition tile
CH = 512            # psum free-dim chunk (fp32 bank limit)
QH = S // 2         # query rows per core
NSUB = D // PT      # 8 feature subtiles
NPAIR = NSUB // 2   # 4 DoubleRow pairs
NKT = S // PT       # 16 k-tiles (full sequence)
NQT = QH // PT      # 8 q-tiles per core
SCALE = 1.0 / float(np.sqrt(H))

S_X, S_M, S_PT = 32.0, 2048.0, 64.0

BF16 = ml_dtypes.bfloat16
F8 = ml_dtypes.float8_e4m3

_NC = None


def _build():
    import concourse.bacc as bacc
    import concourse.mybir as mybir
    from concourse.tile import TileContext

    dt = mybir.dt
    AF = mybir.ActivationFunctionType
    ALU = mybir.AluOpType
    DR = mybir.MatmulPerfMode.DoubleRow

    nc = bacc.Bacc(None, target_bir_lowering=False, num_devices=NCORES,
                   num_swdge_queues=4)

    xrow = nc.declare_dram_parameter("xrow", [PT, NKT, D], dt.bfloat16, isOutput=False)
    wvb = nc.declare_dram_parameter("wvb", [PT, NSUB, H], dt.bfloat16, isOutput=False)
    xq8 = nc.declare_dram_parameter("xq8", [PT, 2, NSUB, CH], dt.float8e4, isOutput=False)
    xf8 = nc.declare_dram_parameter("xf8", [PT, NSUB, S], dt.float8e4, isOutput=False)
    m8 = nc.declare_dram_parameter("m8", [PT, NSUB, NSUB, PT], dt.float8e4, isOutput=False)
    cb = nc.declare_dram_parameter("cb", [PT, NKT], dt.float32, isOutput=False)
    bvb = nc.declare_dram_parameter("bvb", [PT, H], dt.bfloat16, isOutput=False)
    y = nc.declare_dram_parameter("y", [QH, H], dt.float32, isOutput=True)

    with TileContext(nc) as tc:
        with (
            tc.tile_pool(name="pin", bufs=1) as pin,       # persistent inputs
            tc.tile_pool(name="ppt", bufs=1) as ppt,       # PT8
            tc.tile_pool(name="pet", bufs=1) as pet,       # ET (bf16)
            tc.tile_pool(name="pgt", bufs=1) as pgt,       # GT (bf16)
            tc.tile_pool(name="pst", bufs=4) as pst,       # y staging
            tc.tile_pool(name="prd", bufs=2) as prd,
            tc.tile_pool(name="psum", bufs=8, space="PSUM") as pp,
        ):
            def ptile(shape, dtp, tg):
                return pin.tile(shape, dtp, tag=tg, name=tg)

            txr = ptile([PT, NKT, D], dt.bfloat16, "txr")
            twv = ptile([PT, NSUB, H], dt.bfloat16, "twv")
            tx8 = ptile([PT, 2, NSUB, CH], dt.float8e4, "tx8")
            txf = ptile([PT, NSUB, S], dt.float8e4, "txf")
            tm = ptile([PT, NSUB, NSUB, PT], dt.float8e4, "tm")
            tcb = ptile([PT, NKT], dt.float32, "tcb")
            tbv = ptile([PT, H], dt.bfloat16, "tbv")
            tones = ptile([PT, 1], dt.bfloat16, "tones")
            tpt = ppt.tile([PT, NSUB, QH], dt.float8e4, tag="tpt", name="tpt")
            tet = pet.tile([PT, NKT, QH], dt.bfloat16, tag="tet", name="tet")
            tgt = pgt.tile([PT, NSUB, QH], dt.bfloat16, tag="tgt", name="tgt")

            # ---- input loads on one queue, ordered by first use, every DMA
            # a contiguous block (strided column-slices cost 5-10x in issue
            # time and descriptor efficiency). m8/xq8 are host-laid-out so
            # the first PT chunk's operands form contiguous prefixes.
            nc.vector.memset(tones[:], 1.0)
            nc.sync.dma_start(out=tm[:, 0, :, :], in_=m8[:, 0, :, :])
            nc.sync.dma_start(out=tx8[:, 0, :, :], in_=xq8[:, 0, :, :])
            nc.sync.dma_start(out=tm[:, 1, :, :], in_=m8[:, 1, :, :])
            nc.sync.dma_start(out=tm[:, 2:NSUB, :, :], in_=m8[:, 2:NSUB, :, :])
            nc.sync.dma_start(out=tx8[:, 1, :, :], in_=xq8[:, 1, :, :])
            nc.sync.dma_start(out=tcb[:], in_=cb[:, :])
            nc.sync.dma_start(out=txf[:], in_=xf8[:, :, :])
            nc.sync.dma_start(out=txr[:], in_=xrow[:, :, :])
            nc.sync.dma_start(out=twv[:], in_=wvb[:, :, :])
            nc.sync.dma_start(out=tbv[:], in_=bvb[:, :])

            # ---- phase PT+ST, interleaved by q-half so exp starts early ----
            for qc in range(2):
                q0 = qc * CH
                # PT8[d, q] = sum_e M[e,d] x[q,e]  (fp8 DoubleRow, 1-term)
                for dtile in range(NSUB):
                    ps1 = pp.tile([PT, CH], dt.float32, tag="big", name="psb")
                    for j in range(NPAIR):
                        nc.tensor.matmul(
                            ps1[:], tm[:, dtile, 2 * j:2 * j + 2, :],
                            tx8[:, qc, 2 * j:2 * j + 2, :],
                            start=(j == 0), stop=(j == NPAIR - 1), perf_mode=DR)
                    nc.vector.tensor_scalar_mul(
                        tpt[:, dtile, q0:q0 + CH], ps1[:], 2.0 ** -10)
                # ST[k, q] = sum_d x[k,d] PT8[d,q]; ET = exp(2^-16 ST + cb)
                for kt in range(NKT):
                    ps2 = pp.tile([PT, CH], dt.float32, tag="big", name="psb")
                    k0 = kt * PT
                    for j in range(NPAIR):
                        nc.tensor.matmul(
                            ps2[:], txf[:, 2 * j:2 * j + 2, k0:k0 + PT],
                            tpt[:, 2 * j:2 * j + 2, q0:q0 + CH],
                            start=(j == 0), stop=(j == NPAIR - 1), perf_mode=DR)
                    nc.scalar.activation(tet[:, kt, q0:q0 + CH], ps2[:], AF.Exp,
                                         bias=tcb[:, kt:kt + 1], scale=2.0 ** -16)

            # ---- phase G + output, per q-half:
            #   GT[d, q] = sum_k x[k,d] E[k,q]      (bf16, PE layout-native)
            #   den[q]   = sum_k E[k,q]
            #   O[q, h]  = sum_d GT[d,q] Wv[d,h];  y = O*recip(den) + bv
            for qc in range(2):
                q0 = qc * CH
                for dtile in range(NSUB):
                    ps3 = pp.tile([PT, CH], dt.float32, tag="big", name="psb")
                    d0 = dtile * PT
                    for kt in range(NKT):
                        nc.tensor.matmul(ps3[:], txr[:, kt, d0:d0 + PT],
                                         tet[:, kt, q0:q0 + CH],
                                         start=(kt == 0), stop=(kt == NKT - 1))
                    nc.vector.tensor_copy(out=tgt[:, dtile, q0:q0 + CH],
                                          in_=ps3[:])
                for qt in range(4 * qc, 4 * qc + 4):
                    qq = qt * PT
                    dn = pp.tile([PT, 1], dt.float32, tag="big", name="dn")
                    for kt in range(NKT):
                        nc.tensor.matmul(dn[:], tet[:, kt, qq:qq + PT],
                                         tones[:, 0:1],
                                         start=(kt == 0), stop=(kt == NKT - 1))
                    po = [pp.tile([PT, CH], dt.float32, tag="big", name="psb")
                          for _ in range(2)]
                    for dtile in range(NSUB):
                        lg = tgt[:, dtile, qq:qq + PT]
                        for hc in range(2):
                            h0 = hc * CH
                            nc.tensor.matmul(po[hc][:], lg,
                                             twv[:, dtile, h0:h0 + CH],
                                             start=(dtile == 0),
                                             stop=(dtile == NSUB - 1))
                    rd = prd.tile([PT, 1], dt.float32, tag="rd", name="rd")
                    nc.vector.reciprocal(rd[:], dn[:])
                    for hc in range(2):
                        h0 = hc * CH
                        stage = pst.tile([PT, CH], dt.float32, tag="st",
                                         name="stage")
                        nc.vector.scalar_tensor_tensor(
                            stage[:], po[hc][:], rd[:], tbv[:, h0:h0 + CH],
                            ALU.mult, ALU.add)
                        nc.sync.dma_start(out=y[qq:qq + PT, h0:h0 + CH],
                                          in_=stage[:])

    return nc


def _get_nc():
    global _NC
    if _NC is None:
        nc = _build()
        nc.finalize()
        _NC = nc
    return _NC


def _pair_layout(a):
    """[D, N] -> [PT, NSUB, N] with feature subtile on dim1."""
    d, n = a.shape
    return np.ascontiguousarray(a.reshape(NSUB, PT, n).swapaxes(0, 1))


def _prep_inputs(x, Wq, bq, Wk, bk, Wv, bv):
    M = (Wq.astype(np.float64) @ Wk.astype(np.float64).T).astype(np.float32)
    hvec = (Wk.astype(np.float64) @ bq.astype(np.float64)).astype(np.float32)

    # m8[p, dt, es, pd] = M8[es*128+p, dt*128+pd]: the dt-th PT-chunk's
    # stationary operands form a contiguous block.
    m8 = np.ascontiguousarray(
        (M * S_M).astype(F8).reshape(NSUB, PT, NSUB, PT).transpose(1, 2, 0, 3))
    wvb_ = _pair_layout(Wv.astype(BF16))
    bvb = np.ascontiguousarray(np.broadcast_to(bv.astype(BF16), (PT, H)))

    in_maps = []
    for c in range(NCORES):
        b, qh = divmod(c, 2)
        xT = x[b].T.astype(np.float32)  # [D, S]
        x8 = (xT * S_X).astype(F8)
        cbv = (SCALE * (x[b].astype(np.float32) @ hvec)).astype(np.float32)
        q0 = qh * QH
        in_maps.append({
            "xrow": np.ascontiguousarray(
                x[b].astype(BF16).reshape(NKT, PT, D).swapaxes(0, 1)),
            "wvb": wvb_,
            # xq8[p, qc, es, qi] = x8[es*128+p, q0 + qc*512 + qi]
            "xq8": np.ascontiguousarray(
                x8[:, q0:q0 + QH].reshape(NSUB, PT, 2, CH).transpose(1, 2, 0, 3)),
            "xf8": _pair_layout(x8),
            "m8": m8,
            "cb": np.ascontiguousarray(cbv.reshape(NKT, PT).T),
            "bvb": bvb,
        })
    return in_maps


def kernel(x, Wq, bq, Wk, bk, Wv, bv):
    from concourse.bass_utils import run_bass_kernel_spmd

    nc = _get_nc()
    in_maps = _prep_inputs(x, Wq, bq, Wk, bk, Wv, bv)

    trace = bool(os.environ.get("BASS_KERNEL_TRACE"))
    kwargs = {}
    if trace:
        _register_ntff_hook()
        kwargs = {"trace": True, "tmpdir": os.environ.get("BASS_KERNEL_TRACE_DIR")}

    res = run_bass_kernel_spmd(nc, in_maps, list(range(NCORES)), **kwargs)
    if trace:
        kernel.last_exec_time_ns = res.exec_time_ns
        kernel.last_results = res

    out = np.empty((B, S, H), np.float32)
    for c in range(NCORES):
        b, qh = divmod(c, 2)
        out[b, qh * QH:(qh + 1) * QH, :] = res.results[c]["y"]
    return out


def _register_ntff_hook():
    """The container's antenv lacks axon_hooks; register it so trace=True
    can capture NTFF profiles through the axon PJRT library."""
    import sys
    import types

    if "antenv.axon_hooks" in sys.modules:
        return
    mod = types.ModuleType("antenv.axon_hooks")
    holder = [None]
    mod.set_axon_ntff_profile_hook = lambda h: holder.__setitem__(0, h)
    mod.get_axon_ntff_profile_hook = lambda: holder[0]
    sys.modules["antenv.axon_hooks"] = mod
    import antenv

    antenv.axon_hooks = mod
    from trn_agent_boot.trn_boot import _ntff_profile_via_ctypes

    mod.set_axon_ntff_profile_hook(_ntff_profile_via_ctypes("/opt/axon/libaxon_pjrt.so"))


# revision 15
# speedup vs baseline: 1.0168x; 1.0168x over previous
"""Single-head attention (B=4, S=2048, D=H=1024) on 8 TRN2 NeuronCores.

Core c -> batch c//2, query-half c%2 (QH=1024 query rows per core).

Two algebraic restructurings remove both weight applications from the
sequence dimension:

1. scores = Q@K^T = x (Wq Wk^T) x^T + bias terms. With M = Wq Wk^T
   precomputed on host, scores^T[k,q] = (x M x^T)^T + c[k] + (terms
   constant in k, which cancel in softmax). c[k] = x[k]·(Wk bq) is
   host-precomputed and becomes the per-partition bias of the exp
   activation. Kills the K projection entirely.
2. out = (E@V)/den with V = x@Wv + bv  =>  out = (E@x)@Wv/den + bv.
   GT[d,q] = sum_k x[k,d] E[k,q] comes out of the PE in exactly the
   layout the second matmul needs as stationary (no transposes), Wv is
   applied to 1024 q-rows instead of 2048 k-rows, bv folds into the
   final normalize (scalar_tensor_tensor), and no V exchange / no
   collective is needed at all (pair-AllGather measured ~80us
   door-to-done here - far worse than restructuring it away).

fp8 (e4m3) DoubleRow matmuls contract 256/instruction (2x bf16) where
1-term quantization noise fits the 2e-2 gate (numpy bit-sim 1.577e-2,
HW matched sim to ~4e-6 in every round):
  PT8[d,q] = fp8(2^-10 sum_e M8[e,d] xq8[e,q])     fp8 DR   13.7us
  ST[k,q]  = sum_d xf8[d,k] PT8[d,q]               fp8 DR   27.3us
  ET       = exp(2^-16 ST + cb)  (ACT -> bf16)
  GT[d,q]  = sum_k xrow[k,d] ET[k,q]               bf16     54.6us
  O[q,h]   = sum_d GT[d,q] Wv[d,h]                 bf16     27.3us
  den      = ET^T @ ones                           bf16     ~11us
  out      = O*recip(den) + bv                     (DVE STT)
"""

import os

import numpy as np
import ml_dtypes

B, S, D, H = 4, 2048, 1024, 1024
NCORES = 8
PT = 128            # partition tile
CH = 512            # psum free-dim chunk (fp32 bank limit)
QH = S // 2         # query rows per core
NSUB = D // PT      # 8 feature subtiles
NPAIR = NSUB // 2   # 4 DoubleRow pairs
NKT = S // PT       # 16 k-tiles (full sequence)
NQT = QH // PT      # 8 q-tiles per core
SCALE = 1.0 / float(np.sqrt(H))

S_X, S_M, S_PT = 32.0, 2048.0, 64.0

BF16 = ml_dtypes.bfloat16
F8 = ml_dtypes.float8_e4m3

_NC = None


def _build():
    import concourse.bacc as bacc
    import concourse.mybir as mybir
    from concourse.tile import TileContext

    dt = mybir.dt
    AF = mybir.ActivationFunctionType
    ALU = mybir.AluOpType
    DR = mybir.MatmulPerfMode.DoubleRow

    nc = bacc.Bacc(None, target_bir_lowering=False, num_devices=NCORES,
                   num_swdge_queues=4)

    xrow = nc.declare_dram_parameter("xrow", [PT, NKT, D], dt.bfloat16, isOutput=False)
    wvb = nc.declare_dram_parameter("wvb", [PT, NSUB, H], dt.bfloat16, isOutput=False)
    xq8 = nc.declare_dram_parameter("xq8", [PT, 2, NSUB, CH], dt.float8e4, isOutput=False)
    xf8 = nc.declare_dram_parameter("xf8", [PT, NSUB, S], dt.float8e4, isOutput=False)
    m8 = nc.declare_dram_parameter("m8", [PT, NSUB, NSUB, PT], dt.float8e4, isOutput=False)
    cb = nc.declare_dram_parameter("cb", [PT, NKT], dt.float32, isOutput=False)
    bvb = nc.declare_dram_parameter("bvb", [PT, H], dt.bfloat16, isOutput=False)
    y = nc.declare_dram_parameter("y", [QH, H], dt.float32, isOutput=True)

    with TileContext(nc) as tc:
        with (
            tc.tile_pool(name="pin", bufs=1) as pin,       # persistent inputs
            tc.tile_pool(name="ppt", bufs=1) as ppt,       # PT8
            tc.tile_pool(name="pet", bufs=1) as pet,       # ET (bf16)
            tc.tile_pool(name="pgt", bufs=1) as pgt,       # GT (bf16)
            tc.tile_pool(name="pst", bufs=4) as pst,       # y staging
            tc.tile_pool(name="prd", bufs=2) as prd,
            tc.tile_pool(name="psum", bufs=8, space="PSUM") as pp,
        ):
            def ptile(shape, dtp, tg):
                return pin.tile(shape, dtp, tag=tg, name=tg)

            txr = ptile([PT, NKT, D], dt.bfloat16, "txr")
            twv = ptile([PT, NSUB, H], dt.bfloat16, "twv")
            tx8 = ptile([PT, 2, NSUB, CH], dt.float8e4, "tx8")
            txf = ptile([PT, NSUB, S], dt.float8e4, "txf")
            tm = ptile([PT, NSUB, NSUB, PT], dt.float8e4, "tm")
            tcb = ptile([PT, NKT], dt.float32, "tcb")
            tbv = ptile([PT, H], dt.bfloat16, "tbv")
            tones = ptile([PT, 1], dt.bfloat16, "tones")
            tpt = ppt.tile([PT, NSUB, QH], dt.float8e4, tag="tpt", name="tpt")
            tet = pet.tile([PT, NKT, QH], dt.bfloat16, tag="tet", name="tet")
            tgt = pgt.tile([PT, NSUB, QH], dt.bfloat16, tag="tgt", name="tgt")

            # ---- input loads on one queue, ordered by first use, every DMA
            # a contiguous block (strided column-slices cost 5-10x in issue
            # time and descriptor efficiency). m8/xq8 are host-laid-out so
            # the first PT chunk's operands form contiguous prefixes.
            nc.vector.memset(tones[:], 1.0)
            nc.sync.dma_start(out=tm[:, 0, :, :], in_=m8[:, 0, :, :])
            nc.sync.dma_start(out=tx8[:, 0, :, :], in_=xq8[:, 0, :, :])
            nc.sync.dma_start(out=tm[:, 1:NSUB, :, :], in_=m8[:, 1:NSUB, :, :])
            nc.sync.dma_start(out=tx8[:, 1, :, :], in_=xq8[:, 1, :, :])
            nc.sync.dma_start(out=tcb[:], in_=cb[:, :])
            nc.sync.dma_start(out=txf[:], in_=xf8[:, :, :])
            nc.sync.dma_start(out=txr[:], in_=xrow[:, :, :])
            nc.sync.dma_start(out=twv[:], in_=wvb[:, :, :])
            nc.sync.dma_start(out=tbv[:], in_=bvb[:, :])

            # ---- phase PT+ST, interleaved by q-half so exp starts early ----
            for qc in range(2):
                q0 = qc * CH
                # PT8[d, q] = sum_e M[e,d] x[q,e]  (fp8 DoubleRow, 1-term)
                for dtile in range(NSUB):
                    ps1 = pp.tile([PT, CH], dt.float32, tag="big", name="psb")
                    for j in range(NPAIR):
                        nc.tensor.matmul(
                            ps1[:], tm[:, dtile, 2 * j:2 * j + 2, :],
                            tx8[:, qc, 2 * j:2 * j + 2, :],
                            start=(j == 0), stop=(j == NPAIR - 1), perf_mode=DR)
                    nc.vector.tensor_scalar_mul(
                        tpt[:, dtile, q0:q0 + CH], ps1[:], 2.0 ** -10)
                # ST[k, q] = sum_d x[k,d] PT8[d,q]; ET = exp(2^-16 ST + cb)
                for kt in range(NKT):
                    ps2 = pp.tile([PT, CH], dt.float32, tag="big", name="psb")
                    k0 = kt * PT
                    for j in range(NPAIR):
                        nc.tensor.matmul(
                            ps2[:], txf[:, 2 * j:2 * j + 2, k0:k0 + PT],
                            tpt[:, 2 * j:2 * j + 2, q0:q0 + CH],
                            start=(j == 0), stop=(j == NPAIR - 1), perf_mode=DR)
                    nc.scalar.activation(tet[:, kt, q0:q0 + CH], ps2[:], AF.Exp,
                                         bias=tcb[:, kt:kt + 1], scale=2.0 ** -16)

            # ---- phase G + output, per q-half:
            #   GT[d, q] = sum_k x[k,d] E[k,q]      (bf16, PE layout-native)
            #   den[q]   = sum_k E[k,q]
            #   O[q, h]  = sum_d GT[d,q] Wv[d,h];  y = O*recip(den) + bv
            for qc in range(2):
                q0 = qc * CH
                for dtile in range(NSUB):
                    ps3 = pp.tile([PT, CH], dt.float32, tag="big", name="psb")
                    d0 = dtile * PT
                    for kt in range(NKT):
                        nc.tensor.matmul(ps3[:], txr[:, kt, d0:d0 + PT],
                                         tet[:, kt, q0:q0 + CH],
                                         start=(kt == 0), stop=(kt == NKT - 1))
                    nc.vector.tensor_copy(out=tgt[:, dtile, q0:q0 + CH],
                                          in_=ps3[:])
                for qt in range(4 * qc, 4 * qc + 4):
                    qq = qt * PT
                    dn = pp.tile([PT, 1], dt.float32, tag="big", name="dn")
                    for kt in range(NKT):
                        nc.tensor.matmul(dn[:], tet[:, kt, qq:qq + PT],
                                         tones[:, 0:1],
                                         start=(kt == 0), stop=(kt == NKT - 1))
                    po = [pp.tile([PT, CH], dt.float32, tag="big", name="psb")
                          for _ in range(2)]
                    for dtile in range(NSUB):
                        lg = tgt[:, dtile, qq:qq + PT]
                        for hc in range(2):
                            h0 = hc * CH
                            nc.tensor.matmul(po[hc][:], lg,
                                             twv[:, dtile, h0:h0 + CH],
                                             start=(dtile == 0),
                                             stop=(dtile == NSUB - 1))
                    rd = prd.tile([PT, 1], dt.float32, tag="rd", name="rd")
                    nc.vector.reciprocal(rd[:], dn[:])
                    for hc in range(2):
                        h0 = hc * CH
                        stage = pst.tile([PT, CH], dt.float32, tag="st",
                                         name="stage")
                        nc.vector.scalar_tensor_tensor(
                            stage[:], po[hc][:], rd[:], tbv[:, h0:h0 + CH],
                            ALU.mult, ALU.add)
                        nc.sync.dma_start(out=y[qq:qq + PT, h0:h0 + CH],
                                          in_=stage[:])

    return nc


def _get_nc():
    global _NC
    if _NC is None:
        nc = _build()
        nc.finalize()
        _NC = nc
    return _NC


def _pair_layout(a):
    """[D, N] -> [PT, NSUB, N] with feature subtile on dim1."""
    d, n = a.shape
    return np.ascontiguousarray(a.reshape(NSUB, PT, n).swapaxes(0, 1))


def _prep_inputs(x, Wq, bq, Wk, bk, Wv, bv):
    M = (Wq.astype(np.float64) @ Wk.astype(np.float64).T).astype(np.float32)
    hvec = (Wk.astype(np.float64) @ bq.astype(np.float64)).astype(np.float32)

    # m8[p, dt, es, pd] = M8[es*128+p, dt*128+pd]: the dt-th PT-chunk's
    # stationary operands form a contiguous block.
    m8 = np.ascontiguousarray(
        (M * S_M).astype(F8).reshape(NSUB, PT, NSUB, PT).transpose(1, 2, 0, 3))
    wvb_ = _pair_layout(Wv.astype(BF16))
    bvb = np.ascontiguousarray(np.broadcast_to(bv.astype(BF16), (PT, H)))

    in_maps = []
    for c in range(NCORES):
        b, qh = divmod(c, 2)
        xT = x[b].T.astype(np.float32)  # [D, S]
        x8 = (xT * S_X).astype(F8)
        cbv = (SCALE * (x[b].astype(np.float32) @ hvec)).astype(np.float32)
        q0 = qh * QH
        in_maps.append({
            "xrow": np.ascontiguousarray(
                x[b].astype(BF16).reshape(NKT, PT, D).swapaxes(0, 1)),
            "wvb": wvb_,
            # xq8[p, qc, es, qi] = x8[es*128+p, q0 + qc*512 + qi]
            "xq8": np.ascontiguousarray(
                x8[:, q0:q0 + QH].reshape(NSUB, PT, 2, CH).transpose(1, 2, 0, 3)),
            "xf8": _pair_layout(x8),
            "m8": m8,
            "cb": np.ascontiguousarray(cbv.reshape(NKT, PT).T),
            "bvb": bvb,
        })
    return in_maps


def kernel(x, Wq, bq, Wk, bk, Wv, bv):
    from concourse.bass_utils import run_bass_kernel_spmd

    nc = _get_nc()
    in_maps = _prep_inputs(x, Wq, bq, Wk, bk, Wv, bv)

    trace = bool(os.environ.get("BASS_KERNEL_TRACE"))
    kwargs = {}
    if trace:
        _register_ntff_hook()
        kwargs = {"trace": True, "tmpdir": os.environ.get("BASS_KERNEL_TRACE_DIR")}

    res = run_bass_kernel_spmd(nc, in_maps, list(range(NCORES)), **kwargs)
    if trace:
        kernel.last_exec_time_ns = res.exec_time_ns
        kernel.last_results = res

    out = np.empty((B, S, H), np.float32)
    for c in range(NCORES):
        b, qh = divmod(c, 2)
        out[b, qh * QH:(qh + 1) * QH, :] = res.results[c]["y"]
    return out


def _register_ntff_hook():
    """The container's antenv lacks axon_hooks; register it so trace=True
    can capture NTFF profiles through the axon PJRT library."""
    import sys
    import types

    if "antenv.axon_hooks" in sys.modules:
        return
    mod = types.ModuleType("antenv.axon_hooks")
    holder = [None]
    mod.set_axon_ntff_profile_hook = lambda h: holder.__setitem__(0, h)
    mod.get_axon_ntff_profile_hook = lambda: holder[0]
    sys.modules["antenv.axon_hooks"] = mod
    import antenv

    antenv.axon_hooks = mod
    from trn_agent_boot.trn_boot import _ntff_profile_via_ctypes

    mod.set_axon_ntff_profile_hook(_ntff_profile_via_ctypes("/opt/axon/libaxon_pjrt.so"))
